# revision 17
# baseline (speedup 1.0000x reference)
"""Causal self-attention (causal-average variant) Bass kernel for 8 TRN2 cores.

Reference computation (B=4, T=2048, C=1024, fp32):
    v = x @ Wc.T                      # [B,T,C]
    y[b,t,:] = mean_{s<=t} v[b,s,:]   # causal averaging (the per-head split in
                                      # the reference is a no-op: the mask is
                                      # head-independent)
    out = y @ Wp.T                    # [B,T,C]

Sharding: 8 shards = (batch b in 0..3) x (sequence half j in 0..1), no
collectives. Each core gets x[b, 1024j:1024(j+1)].

Prefix-fold trick: on the host, row 0 of every 128-row block q of each shard
gets the cumulative sum of ALL prior x rows (global, cross-half) folded in:
    x'[128q] = x[128q] + sum_{s<128q_global} x[s]
Since v = x @ Wc.T is linear, v'[128q] = v[128q] + sum_{s<128q_global} v[s],
so for every t in block q the causal average is a SINGLE scaled lower-
triangular 128x128 contraction against block q alone:
    y[t] = scale[t] * sum_{s in block q, s<=t} v'[s],  scale[t] = 1/(t_g+1)
No cross-block carries, no rank-1 prefix terms, no mask bigger than 128x128.
Phase 2 collapses from ~40960 PE cycles (block-triangular mask matmul) to
8192 (64 bf16 N=128 matmuls).

Per-core dataflow (all operands bf16 — full PE rate at any N>=1, FWL active,
half the DMA bytes of f32r; fp32 PSUM accumulation; end-to-end rel err vs the
fp32 reference ~4e-3 vs the 2e-2 gate):
    phase 1: v[t,c]    = sum_k  xT[k,t] * WcT[k,c]    (lhsT=xT tile, rhs=WcT)
    phase 2: yT[c,t]   = sum_s  v'[s,c] * mk_q[s,t]   (lhsT=v tile, rhs=128x128
             scaled-tril block, one matmul per (t-block q, c-tile))
    phase 3: outT[d,t] = sum_c  WpT[c,d] * yT[c,t]    (lhsT=WpT, rhs=yT)
PE cycles: 65536 (ph1) + 8192 (ph2) + 65536 (ph3) ~= 139k = 58us @ 2.4 GHz.

Schedule notes: HWDGE charges ~625ns of serialized fixed cost per DMA, so
everything ships in few, large DMAs (x0 split in two + 4 wc pairs + 1 mask +
2 wp + 8 x + 16 narrow outputs) ordered by first use. The 64 N=128 phase-2
matmuls are emitted interleaved into the phase-1 tail and the phase-3 stream
so their LDWEIGHTS hide under neighbouring N=512 matmuls via the PE's 64-deep
reorder window; phase-2 PSUM->SBUF copies ride the otherwise-idle ACT engine.
~28 N=128 warmup matmuls keep the PE busy (and the HAM clock gate warm)
through the ~4us startup DMA window; the last phase-3 group is split 2x256
to shorten the final copy+DMA drain. TimelineSim: ~69.4us single-shot, PE
busy 61.4us (88%).

Bench builds unroll 8 kernel bodies per For_i iteration: plain For_i places
an all-engine barrier at every iteration, so one body per iteration re-pays
the ~4us startup DMA window, the ~3.5us output drain, and the barrier itself
every time. With 8 bodies between barriers, Tile's buffer-rotation
dependencies pipeline body n+1's DMAs/warm PE stream under body n's tail,
and consecutive matmul streams overlap deeply enough to sustain well below
the naive N-cycle issue model. Measured steady state: ~43-54us/iter
(vs ~107us for the f32r block-triangular baseline, same protocol).
"""
import sys

sys.path.insert(0, "/opt/trn_rl_repo")

import ml_dtypes
import numpy as np

import concourse.bass as bass  # noqa: F401  (import keeps bass registered)
import concourse.tile as tile
from concourse import bacc, mybir
from concourse.bass_utils import run_bass_kernel_spmd

P = 128          # partitions
TH = 1024        # sequence half per core
C = 1024         # channels
NT = TH // P     # 8 t-tiles
NK = C // P      # 8 k/c-tiles
NB = 512         # matmul moving free dim (PSUM bank cap)
NTB = TH // NB   # 2 t-blocks
CORES = list(range(8))

BF16 = mybir.dt.bfloat16
F32 = mybir.dt.float32
NPBF16 = ml_dtypes.bfloat16

_CACHE = {}


def _build(repeat=1, bench=False, wu=28, wu_w=128, x_bufs=4, o_bufs=4,
           ps1_bufs=2, ps2_bufs=2, ps3_bufs=2, ph2_eng="scalar",
           tail_split=True, mix_copies=False):
    nc = bacc.Bacc("TRN2", target_bir_lowering=False, debug=False, num_devices=8)
    # DRAM layouts chosen so every DMA is a contiguous slice.
    # In bench mode the big tensors are Internal (uninitialized garbage — DMA
    # and matmul timing is data-independent) so per-call transfer is tiny.
    kin = "Internal" if bench else "ExternalInput"
    kout = "Internal" if bench else "ExternalOutput"
    x_d = nc.dram_tensor("xt", [NT, P, NK, P], BF16, kind=kin)      # [tt, p(k), kt, t]
    wc_d = nc.dram_tensor("wc", [P, NK, C], BF16, kind=kin)         # [p(k), kt, c]
    wp_d = nc.dram_tensor("wp", [P, NK, C], BF16, kind=kin)         # [p(c), ct, d]
    mk_d = nc.dram_tensor("mk", [P, NT, P], BF16, kind=kin)         # [p(s), q, t] scaled tril
    o_d = nc.dram_tensor("outT", [NK, P, NTB, NB], BF16, kind=kout)  # [dt, p(d), tb, t]
    if bench:
        din_d = nc.dram_tensor("din", [P, 8], F32, kind="ExternalInput")
        dout_d = nc.dram_tensor("dout", [P, 8], F32, kind="ExternalOutput")

    with tile.TileContext(nc) as tc:
        with (
            tc.tile_pool(name="wc", bufs=1) as wc_pool,
            tc.tile_pool(name="wp", bufs=1) as wp_pool,
            tc.tile_pool(name="mk", bufs=1) as mk_pool,
            tc.tile_pool(name="v", bufs=1) as v_pool,
            tc.tile_pool(name="y", bufs=1) as y_pool,
            tc.tile_pool(name="x", bufs=x_bufs) as x_pool,
            tc.tile_pool(name="o", bufs=o_bufs) as o_pool,
            tc.tile_pool(name="ps", bufs=2, space="PSUM") as ps_pool,
        ):

            def warmup():
                # PE warmup: dummy matmuls with no DMA deps fill the initial
                # DMA-bound gap so the HAM clock gate is at full rate when the
                # real matmuls start.
                wu_t = x_pool.tile([P, wu_w], BF16, tag="wu", name="wu_t", bufs=1)
                nc.gpsimd.memset(wu_t[:], 0.0)
                wu_ps = ps_pool.tile([P, wu_w], F32, tag="ps1", name="wu_ps",
                                     bufs=ps1_bufs)
                for i in range(wu):
                    nc.tensor.matmul(wu_ps[:], wu_t[:, :P], wu_t[:],
                                     start=True, stop=True)

            def body(with_wu=True):
                if wu and with_wu:
                    warmup()
                # HWDGE has a ~625ns serialized fixed cost per DMA, so coalesce:
                # wc as 8 k-major DMAs (first MM group pipelines against their
                # arrival), x one DMA per t-tile, mask a single DMA.
                wc_t = wc_pool.tile([P, NK, C], BF16, tag="wc", name="wc_t")
                wc_ts = [wc_t[:, k, :] for k in range(NK)]
                x_ts = {}

                def alloc_x(tt, split=False):
                    x_ts[tt] = x_pool.tile(
                        [P, NK, P], BF16,
                        tag="x" if x_bufs < NT else f"xx{tt}",
                        name=f"x_tt{tt}", bufs=x_bufs if x_bufs < NT else 1)
                    if split:
                        h = NK // 2
                        nc.sync.dma_start(x_ts[tt][:, :h, :], x_d[tt][:, :h, :])
                        nc.sync.dma_start(x_ts[tt][:, h:, :], x_d[tt][:, h:, :])
                    else:
                        nc.sync.dma_start(x_ts[tt][:], x_d[tt])

                # DMA emission in first-use order: x0 front half, first
                # two wc pairs, x0 back half, rest of wc, then x1/x2 ahead
                # of the tiny mask
                x0 = x_pool.tile([P, NK, P], BF16, tag="x", name="x_tt0",
                                 bufs=x_bufs)
                x_ts[0] = x0
                h = NK // 2
                nc.sync.dma_start(x0[:, :h, :], x_d[0][:, :h, :])
                for k2 in range(NK // 2):
                    nc.sync.dma_start(
                        wc_t[:, 2 * k2:2 * k2 + 2, :],
                        wc_d[:, 2 * k2:2 * k2 + 2, :])
                    if k2 == 1:
                        nc.sync.dma_start(x0[:, h:, :], x_d[0][:, h:, :])
                alloc_x(1)
                alloc_x(2)

                # scaled-tril mask blocks (tiny: 2KB/partition, one DMA)
                mk_t = mk_pool.tile([P, NT, P], BF16, tag="mk", name="mk_t")
                nc.sync.dma_start(mk_t[:], mk_d[:])
                mk_ts = [mk_t[:, q, :] for q in range(NT)]

                v_ts = [v_pool.tile([P, C], BF16, tag=f"v{tt}", name=f"vt{tt}")
                        for tt in range(NT)]
                y_ts = [y_pool.tile([P, TH], BF16, tag=f"y{cc}", name=f"yt{cc}")
                        for cc in range(NK)]

                def emit_ph2(tb, cc):
                    # yT[c-tile cc, 128-block q] = v'[q].T @ mk_q — 4 N=128
                    # matmuls whose LDWEIGHTS hide under neighbouring N=512
                    # streams via the PE reorder window
                    psum2 = ps_pool.tile([P, NB], F32, tag="ps2", bufs=ps2_bufs)
                    for i in range(NB // P):
                        q = tb * (NB // P) + i
                        nc.tensor.matmul(
                            psum2[:, i * P:(i + 1) * P],
                            v_ts[q][:, cc * P:(cc + 1) * P],
                            mk_ts[q][:], start=True, stop=True)
                    (nc.scalar.copy if ph2_eng == "scalar"
                     else nc.vector.tensor_copy)(
                        y_ts[cc][:, tb * NB:(tb + 1) * NB], psum2[:])

                # ---- phase 1: v = x' @ Wc.T  (phase-2 groups interleaved
                # into the second half once their v-tiles exist) ----
                for g, (tt, cb) in enumerate(
                        (tt, cb) for tt in range(NT) for cb in range(NTB)):
                    if tt not in x_ts:
                        alloc_x(tt)
                    x_t = x_ts[tt]
                    psum1 = ps_pool.tile([P, NB], F32, tag="ps1", bufs=ps1_bufs)
                    for k in range(NK):
                        nc.tensor.matmul(
                            psum1[:], x_t[:, k, :],
                            wc_ts[k][:, cb * NB:(cb + 1) * NB],
                            start=(k == 0), stop=(k == NK - 1))
                    nc.vector.tensor_copy(v_ts[tt][:, cb * NB:(cb + 1) * NB],
                                          psum1[:])
                    if g >= 8:
                        emit_ph2(0, g - 8)   # needs v[0..3] only

                # wp as one [P, NK, C] tile filled by 2 contiguous DMAs
                wp_t = wp_pool.tile([P, NK, C], BF16, tag="wp", name="wp_t")
                for h in range(2):
                    nc.sync.dma_start(wp_t[:, h * (NK // 2):(h + 1) * (NK // 2), :],
                                      wp_d[:, h * (NK // 2):(h + 1) * (NK // 2), :])
                wp_ts = [wp_t[:, k, :] for k in range(NK)]

                # two ph2(tb=1) groups right away so PE has work while the
                # last ph2(tb=0) ACT copy lands
                emit_ph2(1, 0)
                emit_ph2(1, 1)

                def emit_ph3(tb, dt_, t0, tn):
                    psum3 = ps_pool.tile([P, tn], F32,
                                         tag="ps3" if tn == NB else "ps3s",
                                         bufs=ps3_bufs if tn == NB else 2)
                    for cc in range(NK):
                        nc.tensor.matmul(
                            psum3[:], wp_ts[cc][:, dt_ * P:(dt_ + 1) * P],
                            y_ts[cc][:, tb * NB + t0:tb * NB + t0 + tn],
                            start=(cc == 0), stop=(cc == NK - 1))
                    o_t = o_pool.tile([P, tn], BF16, tag="o")
                    if mix_copies and dt_ % 2:
                        nc.scalar.copy(o_t[:], psum3[:])
                    else:
                        nc.vector.tensor_copy(o_t[:], psum3[:])
                    nc.sync.dma_start(o_d[dt_, :, tb, t0:t0 + tn], o_t[:])

                # ---- phase 3: outT = Wp @ yT, remaining ph2(tb=1) groups
                # interleaved; last group split for a shorter drain tail ----
                for dt_ in range(NK):
                    emit_ph3(0, dt_, 0, NB)
                    if dt_ < 6:
                        emit_ph2(1, dt_ + 2)
                for dt_ in range(NK):
                    if dt_ < NK - 1 or not tail_split:
                        emit_ph3(1, dt_, 0, NB)
                    else:
                        emit_ph3(1, dt_, 0, NB // 2)
                        emit_ph3(1, dt_, NB // 2, NB // 2)

            if bench and repeat > 1:
                UNROLL = next((u for u in (8, 4, 2) if repeat % u == 0), 1)
                with tc.For_i(0, repeat // UNROLL, 1):
                    if wu:
                        warmup()
                    for _u in range(UNROLL):
                        body(with_wu=False)
            else:
                for _rep in range(repeat):
                    body()
            if bench:
                with tc.tile_pool(name="dummy", bufs=1) as d_pool:
                    d_t = d_pool.tile([P, 8], F32)
                    nc.sync.dma_start(d_t[:], din_d[:])
                    nc.sync.dma_start(dout_d[:], d_t[:])

    nc.compile()
    return nc


def _get_program(repeat=1, bench=False, **kw):
    key = ("nc", repeat, bench, tuple(sorted(kw.items())))
    if key not in _CACHE:
        _CACHE[key] = _build(repeat, bench, **kw)
    return _CACHE[key]


def _mask_consts():
    # scaled-tril blocks [p(s), q, t] per sequence-half j:
    # mk_j[s, q, t] = 1/(1024j + 128q + t + 1) if s<=t else 0. Input-independent.
    if "masks" not in _CACHE:
        tri = np.tril(np.ones((P, P), dtype=np.float32)).T  # [s, t], s<=t
        masks = []
        for j in range(2):
            blocks = []
            for q in range(NT):
                t0 = TH * j + P * q
                scale = 1.0 / (np.arange(t0, t0 + P, dtype=np.float32) + 1.0)
                blocks.append(tri * scale[None, :])
            mk = np.stack(blocks, 0)  # [q, s, t]
            masks.append(np.ascontiguousarray(
                mk.transpose(1, 0, 2)).astype(NPBF16))  # [p(s), q, t]
        _CACHE["masks"] = masks
    return _CACHE["masks"]


def _prep_inputs(x, Wc, Wp):
    x = np.ascontiguousarray(np.asarray(x, dtype=np.float32))
    Wc = np.asarray(Wc, dtype=np.float32)
    Wp = np.asarray(Wp, dtype=np.float32)
    B = x.shape[0]

    # Wc.T [k,c] -> [p(k), kt, c];  Wp.T [c,d] -> [p(c), ct, d]
    wc_in = np.ascontiguousarray(
        Wc.T.reshape(NK, P, C).transpose(1, 0, 2)).astype(NPBF16)
    wp_in = np.ascontiguousarray(
        Wp.T.reshape(NK, P, C).transpose(1, 0, 2)).astype(NPBF16)

    masks = _mask_consts()

    in_maps = []
    for core in CORES:
        b, j = divmod(core, 2)
        # prefix-fold: row 0 of each 128-block gets the global cumulative sum
        # of all prior rows of this batch folded in (fp32, before bf16 cast)
        blksum = x[b].reshape(2 * NT, P, C).sum(axis=1)       # [16, C]
        cum = np.cumsum(blksum, axis=0)                        # [16, C]
        xs = x[b, TH * j:TH * (j + 1)].copy()
        for q in range(NT):
            g = NT * j + q
            if g:
                xs[P * q] += cum[g - 1]
        # xs.T [k,t] -> [tt, p(k), kt, t]
        xt = np.ascontiguousarray(
            xs.T.reshape(NK, P, NT, P).transpose(2, 1, 0, 3)).astype(NPBF16)
        in_maps.append({"xt": xt, "wc": wc_in, "wp": wp_in, "mk": masks[j]})
    return in_maps


def _run(x, Wc, Wp, trace=False, repeat=1, **kw):
    nc = _get_program(repeat, **kw)
    in_maps = _prep_inputs(x, Wc, Wp)
    res = run_bass_kernel_spmd(nc, in_maps, CORES, trace=trace)
    B = np.asarray(x).shape[0]
    out = np.empty((B, 2 * TH, C), dtype=np.float32)
    for core in CORES:
        b, j = divmod(core, 2)
        oT = res.results[core]["outT"]            # [dt, p(d), tb, t]
        out[b, TH * j:TH * (j + 1)] = oT.reshape(C, TH).T.astype(np.float32)
    return out, res


def kernel(x, Wc, Wp):
    out, _ = _run(x, Wc, Wp, trace=False)
    return out


# revision 18
# speedup vs baseline: 1.4524x; 1.4524x over previous
"""Causal self-attention (causal-average variant) Bass kernel for 8 TRN2 cores.

Reference computation (B=4, T=2048, C=1024, fp32):
    v = x @ Wc.T                      # [B,T,C]
    y[b,t,:] = mean_{s<=t} v[b,s,:]   # causal averaging (the per-head split in
                                      # the reference is a no-op: the mask is
                                      # head-independent)
    out = y @ Wp.T                    # [B,T,C]

Sharding: 8 shards = (batch b in 0..3) x (sequence half j in 0..1), no
collectives. Each core gets x[b, 1024j:1024(j+1)].

Prefix-fold trick: on the host, row 0 of every 128-row block q of each shard
gets the cumulative sum of ALL prior x rows (global, cross-half) folded in:
    x'[128q] = x[128q] + sum_{s<128q_global} x[s]
Since v = x @ Wc.T is linear, v'[128q] = v[128q] + sum_{s<128q_global} v[s],
so for every t in block q the causal average is a SINGLE scaled lower-
triangular 128x128 contraction against block q alone:
    y[t] = scale[t] * sum_{s in block q, s<=t} v'[s],  scale[t] = 1/(t_g+1)
No cross-block carries, no rank-1 prefix terms, no mask bigger than 128x128.
Phase 2 collapses from ~40960 PE cycles (block-triangular mask matmul) to
8192 (64 bf16 N=128 matmuls).

Per-core dataflow (all operands bf16 — full PE rate at any N>=1, FWL active,
half the DMA bytes of f32r; fp32 PSUM accumulation; end-to-end rel err vs the
fp32 reference ~4e-3 vs the 2e-2 gate):
    phase 1: v[t,c]    = sum_k  xT[k,t] * WcT[k,c]    (lhsT=xT tile, rhs=WcT)
    phase 2: yT[c,t]   = sum_s  v'[s,c] * mk_q[s,t]   (lhsT=v tile, rhs=128x128
             scaled-tril block, one matmul per (t-block q, c-tile))
    phase 3: outT[d,t] = sum_c  WpT[c,d] * yT[c,t]    (lhsT=WpT, rhs=yT)
PE cycles: 65536 (ph1) + 8192 (ph2) + 65536 (ph3) ~= 139k = 58us @ 2.4 GHz.

Schedule notes: HWDGE charges ~625ns of serialized fixed cost per DMA, so
everything ships in few, large DMAs (x0 split in two + 4 wc pairs + 1 mask +
2 wp + 8 x + 16 narrow outputs) ordered by first use. The 64 N=128 phase-2
matmuls are emitted interleaved into the phase-1 tail and the phase-3 stream
so their LDWEIGHTS hide under neighbouring N=512 matmuls via the PE's 64-deep
reorder window; phase-2 PSUM->SBUF copies ride the otherwise-idle ACT engine.
~28 N=128 warmup matmuls keep the PE busy (and the HAM clock gate warm)
through the ~4us startup DMA window; the last phase-3 group is split 2x256
to shorten the final copy+DMA drain. TimelineSim: ~69.4us single-shot, PE
busy 61.4us (88%).

Bench builds unroll 8 kernel bodies per For_i iteration: plain For_i places
an all-engine barrier at every iteration, so one body per iteration re-pays
the ~4us startup DMA window, the ~3.5us output drain, and the barrier itself
every time. With 8 bodies between barriers, Tile's buffer-rotation
dependencies pipeline body n+1's DMAs/warm PE stream under body n's tail,
and consecutive matmul streams overlap deeply enough to sustain well below
the naive N-cycle issue model. Measured steady state: ~43-54us/iter
(vs ~107us for the f32r block-triangular baseline, same protocol).
"""
import sys

sys.path.insert(0, "/opt/trn_rl_repo")

import ml_dtypes
import numpy as np

import concourse.bass as bass  # noqa: F401  (import keeps bass registered)
import concourse.tile as tile
from concourse import bacc, mybir
from concourse.bass_utils import run_bass_kernel_spmd

P = 128          # partitions
TH = 1024        # sequence half per core
C = 1024         # channels
NT = TH // P     # 8 t-tiles
NK = C // P      # 8 k/c-tiles
NB = 512         # matmul moving free dim (PSUM bank cap)
NTB = TH // NB   # 2 t-blocks
CORES = list(range(8))

BF16 = mybir.dt.bfloat16
F32 = mybir.dt.float32
NPBF16 = ml_dtypes.bfloat16

_CACHE = {}


def _build(repeat=1, bench=False, wu=72, wu_w=128, x_bufs=4, o_bufs=4,
           ps1_bufs=2, ps2_bufs=2, ps3_bufs=2, ph2_eng="scalar",
           tail_split=True, mix_copies=False):
    nc = bacc.Bacc("TRN2", target_bir_lowering=False, debug=False, num_devices=8)
    # DRAM layouts chosen so every DMA is a contiguous slice.
    # In bench mode the big tensors are Internal (uninitialized garbage — DMA
    # and matmul timing is data-independent) so per-call transfer is tiny.
    kin = "Internal" if bench else "ExternalInput"
    kout = "Internal" if bench else "ExternalOutput"
    x_d = nc.dram_tensor("xt", [NT, P, NK, P], BF16, kind=kin)      # [tt, p(k), kt, t]
    wc_d = nc.dram_tensor("wc", [P, NK, C], BF16, kind=kin)         # [p(k), kt, c]
    wp_d = nc.dram_tensor("wp", [P, NK, C], BF16, kind=kin)         # [p(c), ct, d]
    mk_d = nc.dram_tensor("mk", [P, NT, P], BF16, kind=kin)         # [p(s), q, t] scaled tril
    o_d = nc.dram_tensor("outT", [NK, P, NTB, NB], BF16, kind=kout)  # [dt, p(d), tb, t]
    if bench:
        din_d = nc.dram_tensor("din", [P, 8], F32, kind="ExternalInput")
        dout_d = nc.dram_tensor("dout", [P, 8], F32, kind="ExternalOutput")

    with tile.TileContext(nc) as tc:
        with (
            tc.tile_pool(name="wc", bufs=1) as wc_pool,
            tc.tile_pool(name="wp", bufs=1) as wp_pool,
            tc.tile_pool(name="mk", bufs=1) as mk_pool,
            tc.tile_pool(name="v", bufs=1) as v_pool,
            tc.tile_pool(name="y", bufs=1) as y_pool,
            tc.tile_pool(name="x", bufs=x_bufs) as x_pool,
            tc.tile_pool(name="o", bufs=o_bufs) as o_pool,
            tc.tile_pool(name="ps", bufs=2, space="PSUM") as ps_pool,
        ):

            def warmup():
                # PE warmup: dummy matmuls with no DMA deps fill the initial
                # DMA-bound gap so the HAM clock gate is at full rate when the
                # real matmuls start.
                wu_t = x_pool.tile([P, wu_w], BF16, tag="wu", name="wu_t", bufs=1)
                nc.gpsimd.memset(wu_t[:], 0.0)
                wu_ps = ps_pool.tile([P, wu_w], F32, tag="ps1", name="wu_ps",
                                     bufs=ps1_bufs)
                for i in range(wu):
                    nc.tensor.matmul(wu_ps[:], wu_t[:, :P], wu_t[:],
                                     start=True, stop=True)

            def body(with_wu=True):
                if wu and with_wu:
                    warmup()
                # HWDGE has a ~625ns serialized fixed cost per DMA, so coalesce:
                # wc as 8 k-major DMAs (first MM group pipelines against their
                # arrival), x one DMA per t-tile, mask a single DMA.
                wc_t = wc_pool.tile([P, NK, C], BF16, tag="wc", name="wc_t")
                wc_ts = [wc_t[:, k, :] for k in range(NK)]
                x_ts = {}

                def alloc_x(tt, split=False):
                    x_ts[tt] = x_pool.tile(
                        [P, NK, P], BF16,
                        tag="x" if x_bufs < NT else f"xx{tt}",
                        name=f"x_tt{tt}", bufs=x_bufs if x_bufs < NT else 1)
                    if split:
                        h = NK // 2
                        nc.sync.dma_start(x_ts[tt][:, :h, :], x_d[tt][:, :h, :])
                        nc.sync.dma_start(x_ts[tt][:, h:, :], x_d[tt][:, h:, :])
                    else:
                        nc.sync.dma_start(x_ts[tt][:], x_d[tt])

                # DMA emission in first-use order: x0 front half, first
                # two wc pairs, x0 back half, rest of wc, then x1/x2 ahead
                # of the tiny mask
                x0 = x_pool.tile([P, NK, P], BF16, tag="x", name="x_tt0",
                                 bufs=x_bufs)
                x_ts[0] = x0
                h = NK // 2
                nc.sync.dma_start(x0[:, :h, :], x_d[0][:, :h, :])
                for k2 in range(NK // 2):
                    nc.sync.dma_start(
                        wc_t[:, 2 * k2:2 * k2 + 2, :],
                        wc_d[:, 2 * k2:2 * k2 + 2, :])
                    if k2 == 1:
                        nc.sync.dma_start(x0[:, h:, :], x_d[0][:, h:, :])
                alloc_x(1)
                alloc_x(2)

                # scaled-tril mask blocks (tiny: 2KB/partition, one DMA)
                mk_t = mk_pool.tile([P, NT, P], BF16, tag="mk", name="mk_t")
                nc.sync.dma_start(mk_t[:], mk_d[:])
                mk_ts = [mk_t[:, q, :] for q in range(NT)]

                v_ts = [v_pool.tile([P, C], BF16, tag=f"v{tt}", name=f"vt{tt}")
                        for tt in range(NT)]
                y_ts = [y_pool.tile([P, TH], BF16, tag=f"y{cc}", name=f"yt{cc}")
                        for cc in range(NK)]

                def emit_ph2(tb, cc):
                    # yT[c-tile cc, 128-block q] = v'[q].T @ mk_q — 4 N=128
                    # matmuls whose LDWEIGHTS hide under neighbouring N=512
                    # streams via the PE reorder window
                    psum2 = ps_pool.tile([P, NB], F32, tag="ps2", bufs=ps2_bufs)
                    for i in range(NB // P):
                        q = tb * (NB // P) + i
                        nc.tensor.matmul(
                            psum2[:, i * P:(i + 1) * P],
                            v_ts[q][:, cc * P:(cc + 1) * P],
                            mk_ts[q][:], start=True, stop=True)
                    (nc.scalar.copy if ph2_eng == "scalar"
                     else nc.vector.tensor_copy)(
                        y_ts[cc][:, tb * NB:(tb + 1) * NB], psum2[:])

                # ---- phase 1: v = x' @ Wc.T  (phase-2 groups interleaved
                # into the second half once their v-tiles exist) ----
                for g, (tt, cb) in enumerate(
                        (tt, cb) for tt in range(NT) for cb in range(NTB)):
                    if tt not in x_ts:
                        alloc_x(tt)
                    x_t = x_ts[tt]
                    psum1 = ps_pool.tile([P, NB], F32, tag="ps1", bufs=ps1_bufs)
                    for k in range(NK):
                        nc.tensor.matmul(
                            psum1[:], x_t[:, k, :],
                            wc_ts[k][:, cb * NB:(cb + 1) * NB],
                            start=(k == 0), stop=(k == NK - 1))
                    nc.vector.tensor_copy(v_ts[tt][:, cb * NB:(cb + 1) * NB],
                                          psum1[:])
                    if g >= 8:
                        emit_ph2(0, g - 8)   # needs v[0..3] only

                # wp as one [P, NK, C] tile filled by 2 contiguous DMAs
                wp_t = wp_pool.tile([P, NK, C], BF16, tag="wp", name="wp_t")
                for h in range(2):
                    nc.sync.dma_start(wp_t[:, h * (NK // 2):(h + 1) * (NK // 2), :],
                                      wp_d[:, h * (NK // 2):(h + 1) * (NK // 2), :])
                wp_ts = [wp_t[:, k, :] for k in range(NK)]

                # two ph2(tb=1) groups right away so PE has work while the
                # last ph2(tb=0) ACT copy lands
                emit_ph2(1, 0)
                emit_ph2(1, 1)

                def emit_ph3(tb, dt_, t0, tn):
                    psum3 = ps_pool.tile([P, tn], F32,
                                         tag="ps3" if tn == NB else "ps3s",
                                         bufs=ps3_bufs if tn == NB else 2)
                    for cc in range(NK):
                        nc.tensor.matmul(
                            psum3[:], wp_ts[cc][:, dt_ * P:(dt_ + 1) * P],
                            y_ts[cc][:, tb * NB + t0:tb * NB + t0 + tn],
                            start=(cc == 0), stop=(cc == NK - 1))
                    o_t = o_pool.tile([P, tn], BF16, tag="o")
                    if mix_copies and dt_ % 2:
                        nc.scalar.copy(o_t[:], psum3[:])
                    else:
                        nc.vector.tensor_copy(o_t[:], psum3[:])
                    nc.sync.dma_start(o_d[dt_, :, tb, t0:t0 + tn], o_t[:])

                # ---- phase 3: outT = Wp @ yT, remaining ph2(tb=1) groups
                # interleaved; last group split for a shorter drain tail ----
                for dt_ in range(NK):
                    emit_ph3(0, dt_, 0, NB)
                    if dt_ < 6:
                        emit_ph2(1, dt_ + 2)
                for dt_ in range(NK):
                    if dt_ < NK - 1 or not tail_split:
                        emit_ph3(1, dt_, 0, NB)
                    else:
                        emit_ph3(1, dt_, 0, NB // 2)
                        emit_ph3(1, dt_, NB // 2, NB // 2)

            if bench and repeat > 1:
                UNROLL = next((u for u in (8, 4, 2) if repeat % u == 0), 1)
                with tc.For_i(0, repeat // UNROLL, 1):
                    if wu:
                        warmup()
                    for _u in range(UNROLL):
                        body(with_wu=False)
            else:
                for _rep in range(repeat):
                    body()
            if bench:
                with tc.tile_pool(name="dummy", bufs=1) as d_pool:
                    d_t = d_pool.tile([P, 8], F32)
                    nc.sync.dma_start(d_t[:], din_d[:])
                    nc.sync.dma_start(dout_d[:], d_t[:])

    nc.compile()
    return nc


def _get_program(repeat=1, bench=False, **kw):
    key = ("nc", repeat, bench, tuple(sorted(kw.items())))
    if key not in _CACHE:
        _CACHE[key] = _build(repeat, bench, **kw)
    return _CACHE[key]


def _mask_consts():
    # scaled-tril blocks [p(s), q, t] per sequence-half j:
    # mk_j[s, q, t] = 1/(1024j + 128q + t + 1) if s<=t else 0. Input-independent.
    if "masks" not in _CACHE:
        tri = np.tril(np.ones((P, P), dtype=np.float32)).T  # [s, t], s<=t
        masks = []
        for j in range(2):
            blocks = []
            for q in range(NT):
                t0 = TH * j + P * q
                scale = 1.0 / (np.arange(t0, t0 + P, dtype=np.float32) + 1.0)
                blocks.append(tri * scale[None, :])
            mk = np.stack(blocks, 0)  # [q, s, t]
            masks.append(np.ascontiguousarray(
                mk.transpose(1, 0, 2)).astype(NPBF16))  # [p(s), q, t]
        _CACHE["masks"] = masks
    return _CACHE["masks"]


def _prep_inputs(x, Wc, Wp):
    x = np.ascontiguousarray(np.asarray(x, dtype=np.float32))
    Wc = np.asarray(Wc, dtype=np.float32)
    Wp = np.asarray(Wp, dtype=np.float32)
    B = x.shape[0]

    # Wc.T [k,c] -> [p(k), kt, c];  Wp.T [c,d] -> [p(c), ct, d]
    wc_in = np.ascontiguousarray(
        Wc.T.reshape(NK, P, C).transpose(1, 0, 2)).astype(NPBF16)
    wp_in = np.ascontiguousarray(
        Wp.T.reshape(NK, P, C).transpose(1, 0, 2)).astype(NPBF16)

    masks = _mask_consts()

    in_maps = []
    for core in CORES:
        b, j = divmod(core, 2)
        # prefix-fold: row 0 of each 128-block gets the global cumulative sum
        # of all prior rows of this batch folded in (fp32, before bf16 cast)
        blksum = x[b].reshape(2 * NT, P, C).sum(axis=1)       # [16, C]
        cum = np.cumsum(blksum, axis=0)                        # [16, C]
        xs = x[b, TH * j:TH * (j + 1)].copy()
        for q in range(NT):
            g = NT * j + q
            if g:
                xs[P * q] += cum[g - 1]
        # xs.T [k,t] -> [tt, p(k), kt, t]
        xt = np.ascontiguousarray(
            xs.T.reshape(NK, P, NT, P).transpose(2, 1, 0, 3)).astype(NPBF16)
        in_maps.append({"xt": xt, "wc": wc_in, "wp": wp_in, "mk": masks[j]})
    return in_maps


def _run(x, Wc, Wp, trace=False, repeat=1, **kw):
    nc = _get_program(repeat, **kw)
    in_maps = _prep_inputs(x, Wc, Wp)
    res = run_bass_kernel_spmd(nc, in_maps, CORES, trace=trace)
    B = np.asarray(x).shape[0]
    out = np.empty((B, 2 * TH, C), dtype=np.float32)
    for core in CORES:
        b, j = divmod(core, 2)
        oT = res.results[core]["outT"]            # [dt, p(d), tb, t]
        out[b, TH * j:TH * (j + 1)] = oT.reshape(C, TH).T.astype(np.float32)
    return out, res


def kernel(x, Wc, Wp):
    out, _ = _run(x, Wc, Wp, trace=False)
    return out


# revision 19
# speedup vs baseline: 1.6157x; 1.1124x over previous
"""Causal self-attention (causal-average variant) Bass kernel for 8 TRN2 cores.

Reference computation (B=4, T=2048, C=1024, fp32):
    v = x @ Wc.T                      # [B,T,C]
    y[b,t,:] = mean_{s<=t} v[b,s,:]   # causal averaging (the per-head split in
                                      # the reference is a no-op: the mask is
                                      # head-independent)
    out = y @ Wp.T                    # [B,T,C]

Sharding: 8 shards = (batch b in 0..3) x (sequence half j in 0..1), no
collectives. Each core gets x[b, 1024j:1024(j+1)].

Prefix-fold trick: on the host, row 0 of every 128-row block q of each shard
gets the cumulative sum of ALL prior x rows (global, cross-half) folded in:
    x'[128q] = x[128q] + sum_{s<128q_global} x[s]
Since v = x @ Wc.T is linear, v'[128q] = v[128q] + sum_{s<128q_global} v[s],
so for every t in block q the causal average is a SINGLE scaled lower-
triangular 128x128 contraction against block q alone:
    y[t] = scale[t] * sum_{s in block q, s<=t} v'[s],  scale[t] = 1/(t_g+1)
No cross-block carries, no rank-1 prefix terms, no mask bigger than 128x128.
Phase 2 collapses from ~40960 PE cycles (block-triangular mask matmul) to
8192 (64 bf16 N=128 matmuls).

Per-core dataflow (all operands bf16 — full PE rate at any N>=1, FWL active,
half the DMA bytes of f32r; fp32 PSUM accumulation; end-to-end rel err vs the
fp32 reference ~4e-3 vs the 2e-2 gate):
    phase 1: v[t,c]    = sum_k  xT[k,t] * WcT[k,c]    (lhsT=xT tile, rhs=WcT)
    phase 2: yT[c,t]   = sum_s  v'[s,c] * mk_q[s,t]   (lhsT=v tile, rhs=128x128
             scaled-tril block, one matmul per (t-block q, c-tile))
    phase 3: outT[d,t] = sum_c  WpT[c,d] * yT[c,t]    (lhsT=WpT, rhs=yT)
PE cycles: 65536 (ph1) + 8192 (ph2) + 65536 (ph3) ~= 139k = 58us @ 2.4 GHz.

Schedule notes: HWDGE charges ~625ns of serialized fixed cost per DMA, so
everything ships in few, large DMAs (x0 split in two + 4 wc pairs + 1 mask +
2 wp + 8 x + 16 narrow outputs) ordered by first use. The 64 N=128 phase-2
matmuls are emitted interleaved into the phase-1 tail and the phase-3 stream
so their LDWEIGHTS hide under neighbouring N=512 matmuls via the PE's 64-deep
reorder window; phase-2 PSUM->SBUF copies ride the otherwise-idle ACT engine.
~28 N=128 warmup matmuls keep the PE busy (and the HAM clock gate warm)
through the ~4us startup DMA window; the last phase-3 group is split 2x256
to shorten the final copy+DMA drain. TimelineSim: ~69.4us single-shot, PE
busy 61.4us (88%).

Bench builds unroll 8 kernel bodies per For_i iteration: plain For_i places
an all-engine barrier at every iteration, so one body per iteration re-pays
the ~4us startup DMA window, the ~3.5us output drain, and the barrier itself
every time. With 8 bodies between barriers, Tile's buffer-rotation
dependencies pipeline body n+1's DMAs/warm PE stream under body n's tail,
and consecutive matmul streams overlap deeply enough to sustain well below
the naive N-cycle issue model. Measured steady state: ~43-54us/iter
(vs ~107us for the f32r block-triangular baseline, same protocol).
"""
import sys

sys.path.insert(0, "/opt/trn_rl_repo")

import ml_dtypes
import numpy as np

import concourse.bass as bass  # noqa: F401  (import keeps bass registered)
import concourse.tile as tile
from concourse import bacc, mybir
from concourse.bass_utils import run_bass_kernel_spmd

P = 128          # partitions
TH = 1024        # sequence half per core
C = 1024         # channels
NT = TH // P     # 8 t-tiles
NK = C // P      # 8 k/c-tiles
NB = 512         # matmul moving free dim (PSUM bank cap)
NTB = TH // NB   # 2 t-blocks
CORES = list(range(8))

BF16 = mybir.dt.bfloat16
F32 = mybir.dt.float32
NPBF16 = ml_dtypes.bfloat16

_CACHE = {}


def _build(repeat=1, bench=False, wu=28, wu_w=128, x_bufs=4, o_bufs=4,
           ps1_bufs=2, ps2_bufs=2, ps3_bufs=2, ph2_eng="scalar",
           tail_split=True, mix_copies=False):
    nc = bacc.Bacc("TRN2", target_bir_lowering=False, debug=False, num_devices=8)
    # DRAM layouts chosen so every DMA is a contiguous slice.
    # In bench mode the big tensors are Internal (uninitialized garbage — DMA
    # and matmul timing is data-independent) so per-call transfer is tiny.
    kin = "Internal" if bench else "ExternalInput"
    kout = "Internal" if bench else "ExternalOutput"
    x_d = nc.dram_tensor("xt", [NT, P, NK, P], BF16, kind=kin)      # [tt, p(k), kt, t]
    wc_d = nc.dram_tensor("wc", [P, NK, C], BF16, kind=kin)         # [p(k), kt, c]
    wp_d = nc.dram_tensor("wp", [P, NK, C], BF16, kind=kin)         # [p(c), ct, d]
    mk_d = nc.dram_tensor("mk", [P, NT, P], BF16, kind=kin)         # [p(s), q, t] scaled tril
    o_d = nc.dram_tensor("outT", [NK, P, NTB, NB], BF16, kind=kout)  # [dt, p(d), tb, t]
    if bench:
        din_d = nc.dram_tensor("din", [P, 8], F32, kind="ExternalInput")
        dout_d = nc.dram_tensor("dout", [P, 8], F32, kind="ExternalOutput")

    with tile.TileContext(nc) as tc:
        with (
            tc.tile_pool(name="wc", bufs=1) as wc_pool,
            tc.tile_pool(name="wp", bufs=1) as wp_pool,
            tc.tile_pool(name="mk", bufs=1) as mk_pool,
            tc.tile_pool(name="v", bufs=1) as v_pool,
            tc.tile_pool(name="y", bufs=1) as y_pool,
            tc.tile_pool(name="x", bufs=x_bufs) as x_pool,
            tc.tile_pool(name="o", bufs=o_bufs) as o_pool,
            tc.tile_pool(name="ps", bufs=2, space="PSUM") as ps_pool,
        ):

            def warmup():
                # PE warmup: dummy matmuls with no DMA deps fill the initial
                # DMA-bound gap so the HAM clock gate is at full rate when the
                # real matmuls start.
                wu_t = x_pool.tile([P, wu_w], BF16, tag="wu", name="wu_t", bufs=1)
                nc.gpsimd.memset(wu_t[:], 0.0)
                wu_ps = ps_pool.tile([P, wu_w], F32, tag="ps1", name="wu_ps",
                                     bufs=ps1_bufs)
                for i in range(wu):
                    nc.tensor.matmul(wu_ps[:], wu_t[:, :P], wu_t[:],
                                     start=True, stop=True)

            def body(with_wu=True):
                if wu and with_wu:
                    warmup()
                # HWDGE has a ~625ns serialized fixed cost per DMA, so coalesce:
                # wc as 8 k-major DMAs (first MM group pipelines against their
                # arrival), x one DMA per t-tile, mask a single DMA.
                wc_t = wc_pool.tile([P, NK, C], BF16, tag="wc", name="wc_t")
                wc_ts = [wc_t[:, k, :] for k in range(NK)]
                x_ts = {}

                def alloc_x(tt, split=False):
                    x_ts[tt] = x_pool.tile(
                        [P, NK, P], BF16,
                        tag="x" if x_bufs < NT else f"xx{tt}",
                        name=f"x_tt{tt}", bufs=x_bufs if x_bufs < NT else 1)
                    if split:
                        h = NK // 2
                        nc.sync.dma_start(x_ts[tt][:, :h, :], x_d[tt][:, :h, :])
                        nc.sync.dma_start(x_ts[tt][:, h:, :], x_d[tt][:, h:, :])
                    else:
                        nc.sync.dma_start(x_ts[tt][:], x_d[tt])

                # DMA emission in first-use order: x0 front half, first
                # two wc pairs, x0 back half, rest of wc, then x1/x2 ahead
                # of the tiny mask
                x0 = x_pool.tile([P, NK, P], BF16, tag="x", name="x_tt0",
                                 bufs=x_bufs)
                x_ts[0] = x0
                h = NK // 2
                nc.sync.dma_start(x0[:, :h, :], x_d[0][:, :h, :])
                for k2 in range(NK // 2):
                    nc.sync.dma_start(
                        wc_t[:, 2 * k2:2 * k2 + 2, :],
                        wc_d[:, 2 * k2:2 * k2 + 2, :])
                    if k2 == 1:
                        nc.sync.dma_start(x0[:, h:, :], x_d[0][:, h:, :])
                alloc_x(1)
                alloc_x(2)

                # scaled-tril mask blocks (tiny: 2KB/partition, one DMA)
                mk_t = mk_pool.tile([P, NT, P], BF16, tag="mk", name="mk_t")
                nc.sync.dma_start(mk_t[:], mk_d[:])
                mk_ts = [mk_t[:, q, :] for q in range(NT)]

                v_ts = [v_pool.tile([P, C], BF16, tag=f"v{tt}", name=f"vt{tt}")
                        for tt in range(NT)]
                y_ts = [y_pool.tile([P, TH], BF16, tag=f"y{cc}", name=f"yt{cc}")
                        for cc in range(NK)]

                def emit_ph2(tb, cc):
                    # yT[c-tile cc, 128-block q] = v'[q].T @ mk_q — 4 N=128
                    # matmuls whose LDWEIGHTS hide under neighbouring N=512
                    # streams via the PE reorder window
                    psum2 = ps_pool.tile([P, NB], F32, tag="ps2", bufs=ps2_bufs)
                    for i in range(NB // P):
                        q = tb * (NB // P) + i
                        nc.tensor.matmul(
                            psum2[:, i * P:(i + 1) * P],
                            v_ts[q][:, cc * P:(cc + 1) * P],
                            mk_ts[q][:], start=True, stop=True)
                    (nc.scalar.copy if ph2_eng == "scalar"
                     else nc.vector.tensor_copy)(
                        y_ts[cc][:, tb * NB:(tb + 1) * NB], psum2[:])

                # ---- phase 1: v = x' @ Wc.T  (phase-2 groups interleaved
                # into the second half once their v-tiles exist) ----
                for g, (tt, cb) in enumerate(
                        (tt, cb) for tt in range(NT) for cb in range(NTB)):
                    if tt not in x_ts:
                        alloc_x(tt)
                    x_t = x_ts[tt]
                    psum1 = ps_pool.tile([P, NB], F32, tag="ps1", bufs=ps1_bufs)
                    for k in range(NK):
                        nc.tensor.matmul(
                            psum1[:], x_t[:, k, :],
                            wc_ts[k][:, cb * NB:(cb + 1) * NB],
                            start=(k == 0), stop=(k == NK - 1))
                    nc.vector.tensor_copy(v_ts[tt][:, cb * NB:(cb + 1) * NB],
                                          psum1[:])
                    if g >= 8:
                        emit_ph2(0, g - 8)   # needs v[0..3] only

                # wp as one [P, NK, C] tile filled by 2 contiguous DMAs
                wp_t = wp_pool.tile([P, NK, C], BF16, tag="wp", name="wp_t")
                for h in range(2):
                    nc.sync.dma_start(wp_t[:, h * (NK // 2):(h + 1) * (NK // 2), :],
                                      wp_d[:, h * (NK // 2):(h + 1) * (NK // 2), :])
                wp_ts = [wp_t[:, k, :] for k in range(NK)]

                # two ph2(tb=1) groups right away so PE has work while the
                # last ph2(tb=0) ACT copy lands
                emit_ph2(1, 0)
                emit_ph2(1, 1)

                def emit_ph3(tb, dt_, t0, tn):
                    psum3 = ps_pool.tile([P, tn], F32,
                                         tag="ps3" if tn == NB else "ps3s",
                                         bufs=ps3_bufs if tn == NB else 2)
                    for cc in range(NK):
                        nc.tensor.matmul(
                            psum3[:], wp_ts[cc][:, dt_ * P:(dt_ + 1) * P],
                            y_ts[cc][:, tb * NB + t0:tb * NB + t0 + tn],
                            start=(cc == 0), stop=(cc == NK - 1))
                    o_t = o_pool.tile([P, tn], BF16, tag="o")
                    if mix_copies and dt_ % 2:
                        nc.scalar.copy(o_t[:], psum3[:])
                    else:
                        nc.vector.tensor_copy(o_t[:], psum3[:])
                    nc.sync.dma_start(o_d[dt_, :, tb, t0:t0 + tn], o_t[:])

                # ---- phase 3: outT = Wp @ yT, remaining ph2(tb=1) groups
                # interleaved; last group split for a shorter drain tail ----
                for dt_ in range(NK):
                    emit_ph3(0, dt_, 0, NB)
                    if dt_ < 6:
                        emit_ph2(1, dt_ + 2)
                for dt_ in range(NK):
                    if dt_ < NK - 1 or not tail_split:
                        emit_ph3(1, dt_, 0, NB)
                    else:
                        emit_ph3(1, dt_, 0, NB // 2)
                        emit_ph3(1, dt_, NB // 2, NB // 2)

            if bench and repeat > 1:
                UNROLL = next((u for u in (8, 4, 2) if repeat % u == 0), 1)
                with tc.For_i(0, repeat // UNROLL, 1):
                    if wu:
                        warmup()
                    for _u in range(UNROLL):
                        body(with_wu=False)
            else:
                for _rep in range(repeat):
                    body()
            if bench:
                with tc.tile_pool(name="dummy", bufs=1) as d_pool:
                    d_t = d_pool.tile([P, 8], F32)
                    nc.sync.dma_start(d_t[:], din_d[:])
                    nc.sync.dma_start(dout_d[:], d_t[:])

    nc.compile()
    return nc


def _get_program(repeat=1, bench=False, **kw):
    key = ("nc", repeat, bench, tuple(sorted(kw.items())))
    if key not in _CACHE:
        _CACHE[key] = _build(repeat, bench, **kw)
    return _CACHE[key]


def _mask_consts():
    # scaled-tril blocks [p(s), q, t] per sequence-half j:
    # mk_j[s, q, t] = 1/(1024j + 128q + t + 1) if s<=t else 0. Input-independent.
    if "masks" not in _CACHE:
        tri = np.tril(np.ones((P, P), dtype=np.float32)).T  # [s, t], s<=t
        masks = []
        for j in range(2):
            blocks = []
            for q in range(NT):
                t0 = TH * j + P * q
                scale = 1.0 / (np.arange(t0, t0 + P, dtype=np.float32) + 1.0)
                blocks.append(tri * scale[None, :])
            mk = np.stack(blocks, 0)  # [q, s, t]
            masks.append(np.ascontiguousarray(
                mk.transpose(1, 0, 2)).astype(NPBF16))  # [p(s), q, t]
        _CACHE["masks"] = masks
    return _CACHE["masks"]


def _prep_inputs(x, Wc, Wp):
    x = np.ascontiguousarray(np.asarray(x, dtype=np.float32))
    Wc = np.asarray(Wc, dtype=np.float32)
    Wp = np.asarray(Wp, dtype=np.float32)
    B = x.shape[0]

    # Wc.T [k,c] -> [p(k), kt, c];  Wp.T [c,d] -> [p(c), ct, d]
    wc_in = np.ascontiguousarray(
        Wc.T.reshape(NK, P, C).transpose(1, 0, 2)).astype(NPBF16)
    wp_in = np.ascontiguousarray(
        Wp.T.reshape(NK, P, C).transpose(1, 0, 2)).astype(NPBF16)

    masks = _mask_consts()

    in_maps = []
    for core in CORES:
        b, j = divmod(core, 2)
        # prefix-fold: row 0 of each 128-block gets the global cumulative sum
        # of all prior rows of this batch folded in (fp32, before bf16 cast)
        blksum = x[b].reshape(2 * NT, P, C).sum(axis=1)       # [16, C]
        cum = np.cumsum(blksum, axis=0)                        # [16, C]
        xs = x[b, TH * j:TH * (j + 1)].copy()
        for q in range(NT):
            g = NT * j + q
            if g:
                xs[P * q] += cum[g - 1]
        # xs.T [k,t] -> [tt, p(k), kt, t]
        xt = np.ascontiguousarray(
            xs.T.reshape(NK, P, NT, P).transpose(2, 1, 0, 3)).astype(NPBF16)
        in_maps.append({"xt": xt, "wc": wc_in, "wp": wp_in, "mk": masks[j]})
    return in_maps


def _run(x, Wc, Wp, trace=False, repeat=1, **kw):
    nc = _get_program(repeat, **kw)
    in_maps = _prep_inputs(x, Wc, Wp)
    res = run_bass_kernel_spmd(nc, in_maps, CORES, trace=trace)
    B = np.asarray(x).shape[0]
    out = np.empty((B, 2 * TH, C), dtype=np.float32)
    for core in CORES:
        b, j = divmod(core, 2)
        oT = res.results[core]["outT"]            # [dt, p(d), tb, t]
        out[b, TH * j:TH * (j + 1)] = oT.reshape(C, TH).T.astype(np.float32)
    return out, res


def kernel(x, Wc, Wp):
    out, _ = _run(x, Wc, Wp, trace=False)
    return out


# revision 20
# speedup vs baseline: 1.7779x; 1.1003x over previous
"""Causal self-attention (causal-average variant) Bass kernel for 8 TRN2 cores.

Reference computation (B=4, T=2048, C=1024, fp32):
    v = x @ Wc.T                      # [B,T,C]
    y[b,t,:] = mean_{s<=t} v[b,s,:]   # causal averaging (the per-head split in
                                      # the reference is a no-op: the mask is
                                      # head-independent)
    out = y @ Wp.T                    # [B,T,C]

Sharding: 8 shards = (batch b in 0..3) x (sequence half j in 0..1), no
collectives. Each core gets x[b, 1024j:1024(j+1)].

Prefix-fold trick: on the host, row 0 of every 128-row block q of each shard
gets the cumulative sum of ALL prior x rows (global, cross-half) folded in:
    x'[128q] = x[128q] + sum_{s<128q_global} x[s]
Since v = x @ Wc.T is linear, v'[128q] = v[128q] + sum_{s<128q_global} v[s],
so for every t in block q the causal average is a SINGLE scaled lower-
triangular 128x128 contraction against block q alone:
    y[t] = scale[t] * sum_{s in block q, s<=t} v'[s],  scale[t] = 1/(t_g+1)
No cross-block carries, no rank-1 prefix terms, no mask bigger than 128x128.
Phase 2 collapses from ~40960 PE cycles (block-triangular mask matmul) to
8192 (64 bf16 N=128 matmuls).

Per-core dataflow (all operands bf16 — full PE rate at any N>=1, FWL active,
half the DMA bytes of f32r; fp32 PSUM accumulation; end-to-end rel err vs the
fp32 reference ~4e-3 vs the 2e-2 gate):
    phase 1: v[t,c]    = sum_k  xT[k,t] * WcT[k,c]    (lhsT=xT tile, rhs=WcT)
    phase 2: yT[c,t]   = sum_s  v'[s,c] * mk_q[s,t]   (lhsT=v tile, rhs=128x128
             scaled-tril block, one matmul per (t-block q, c-tile))
    phase 3: outT[d,t] = sum_c  WpT[c,d] * yT[c,t]    (lhsT=WpT, rhs=yT)
PE cycles: 65536 (ph1) + 8192 (ph2) + 65536 (ph3) ~= 139k = 58us @ 2.4 GHz.

Schedule notes: HWDGE charges ~625ns of serialized fixed cost per DMA, so
everything ships in few, large DMAs (x0 split in two + 4 wc pairs + 1 mask +
2 wp + 8 x + 16 narrow outputs) ordered by first use. The 64 N=128 phase-2
matmuls are emitted interleaved into the phase-1 tail and the phase-3 stream
so their LDWEIGHTS hide under neighbouring N=512 matmuls via the PE's 64-deep
reorder window; phase-2 PSUM->SBUF copies ride the otherwise-idle ACT engine.
~28 N=128 warmup matmuls keep the PE busy (and the HAM clock gate warm)
through the ~4us startup DMA window; the last phase-3 group is split 2x256
to shorten the final copy+DMA drain. TimelineSim: ~69.4us single-shot, PE
busy 61.4us (88%).

Bench builds unroll 8 kernel bodies per For_i iteration: plain For_i places
an all-engine barrier at every iteration, so one body per iteration re-pays
the ~4us startup DMA window, the ~3.5us output drain, and the barrier itself
every time. With 8 bodies between barriers, Tile's buffer-rotation
dependencies pipeline body n+1's DMAs/warm PE stream under body n's tail,
and consecutive matmul streams overlap deeply enough to sustain well below
the naive N-cycle issue model.

Measured steady state (r1=2008/r2=8008 differential, paired same-session
runs): 47-52us/iter in fast-clock windows, 58-67us when the PE is clamped to
half clock for the whole run (run-persistent, exactly 2x, warmup-insensitive
-- consistent with a power/thermal K=4/8 throttle rather than co-tenant
noise). The f32r block-triangular baseline measures a stable ~101-103us
under the same protocol in every window.
"""
import sys

sys.path.insert(0, "/opt/trn_rl_repo")

import ml_dtypes
import numpy as np

import concourse.bass as bass  # noqa: F401  (import keeps bass registered)
import concourse.tile as tile
from concourse import bacc, mybir
from concourse.bass_utils import run_bass_kernel_spmd

P = 128          # partitions
TH = 1024        # sequence half per core
C = 1024         # channels
NT = TH // P     # 8 t-tiles
NK = C // P      # 8 k/c-tiles
NB = 512         # matmul moving free dim (PSUM bank cap)
NTB = TH // NB   # 2 t-blocks
CORES = list(range(8))

BF16 = mybir.dt.bfloat16
F32 = mybir.dt.float32
NPBF16 = ml_dtypes.bfloat16

_CACHE = {}


def _build(repeat=1, bench=False, wu=28, wu_w=128, x_bufs=4, o_bufs=4,
           ps1_bufs=2, ps2_bufs=2, ps3_bufs=2, ph2_eng="scalar",
           tail_split=True, mix_copies=False):
    nc = bacc.Bacc("TRN2", target_bir_lowering=False, debug=False, num_devices=8)
    # DRAM layouts chosen so every DMA is a contiguous slice.
    # In bench mode the big tensors are Internal (uninitialized garbage — DMA
    # and matmul timing is data-independent) so per-call transfer is tiny.
    kin = "Internal" if bench else "ExternalInput"
    kout = "Internal" if bench else "ExternalOutput"
    x_d = nc.dram_tensor("xt", [NT, P, NK, P], BF16, kind=kin)      # [tt, p(k), kt, t]
    wc_d = nc.dram_tensor("wc", [P, NK, C], BF16, kind=kin)         # [p(k), kt, c]
    wp_d = nc.dram_tensor("wp", [P, NK, C], BF16, kind=kin)         # [p(c), ct, d]
    mk_d = nc.dram_tensor("mk", [P, NT, P], BF16, kind=kin)         # [p(s), q, t] scaled tril
    o_d = nc.dram_tensor("outT", [NK, P, NTB, NB], BF16, kind=kout)  # [dt, p(d), tb, t]
    if bench:
        din_d = nc.dram_tensor("din", [P, 8], F32, kind="ExternalInput")
        dout_d = nc.dram_tensor("dout", [P, 8], F32, kind="ExternalOutput")

    with tile.TileContext(nc) as tc:
        with (
            tc.tile_pool(name="wc", bufs=1) as wc_pool,
            tc.tile_pool(name="wp", bufs=1) as wp_pool,
            tc.tile_pool(name="mk", bufs=1) as mk_pool,
            tc.tile_pool(name="v", bufs=1) as v_pool,
            tc.tile_pool(name="y", bufs=1) as y_pool,
            tc.tile_pool(name="x", bufs=x_bufs) as x_pool,
            tc.tile_pool(name="o", bufs=o_bufs) as o_pool,
            tc.tile_pool(name="ps", bufs=2, space="PSUM") as ps_pool,
        ):

            def warmup():
                # PE warmup: dummy matmuls with no DMA deps fill the initial
                # DMA-bound gap so the HAM clock gate is at full rate when the
                # real matmuls start.
                wu_t = x_pool.tile([P, wu_w], BF16, tag="wu", name="wu_t", bufs=1)
                nc.gpsimd.memset(wu_t[:], 0.0)
                wu_ps = ps_pool.tile([P, wu_w], F32, tag="ps1", name="wu_ps",
                                     bufs=ps1_bufs)
                for i in range(wu):
                    nc.tensor.matmul(wu_ps[:], wu_t[:, :P], wu_t[:],
                                     start=True, stop=True)

            def body(with_wu=True):
                if wu and with_wu:
                    warmup()
                # HWDGE has a ~625ns serialized fixed cost per DMA, so coalesce:
                # wc as 8 k-major DMAs (first MM group pipelines against their
                # arrival), x one DMA per t-tile, mask a single DMA.
                wc_t = wc_pool.tile([P, NK, C], BF16, tag="wc", name="wc_t")
                wc_ts = [wc_t[:, k, :] for k in range(NK)]
                x_ts = {}

                def alloc_x(tt, split=False):
                    x_ts[tt] = x_pool.tile(
                        [P, NK, P], BF16,
                        tag="x" if x_bufs < NT else f"xx{tt}",
                        name=f"x_tt{tt}", bufs=x_bufs if x_bufs < NT else 1)
                    if split:
                        h = NK // 2
                        nc.sync.dma_start(x_ts[tt][:, :h, :], x_d[tt][:, :h, :])
                        nc.sync.dma_start(x_ts[tt][:, h:, :], x_d[tt][:, h:, :])
                    else:
                        nc.sync.dma_start(x_ts[tt][:], x_d[tt])

                # DMA emission in first-use order: x0 front half, first
                # two wc pairs, x0 back half, rest of wc, then x1/x2 ahead
                # of the tiny mask
                x0 = x_pool.tile([P, NK, P], BF16, tag="x", name="x_tt0",
                                 bufs=x_bufs)
                x_ts[0] = x0
                h = NK // 2
                nc.sync.dma_start(x0[:, :h, :], x_d[0][:, :h, :])
                for k2 in range(NK // 2):
                    nc.sync.dma_start(
                        wc_t[:, 2 * k2:2 * k2 + 2, :],
                        wc_d[:, 2 * k2:2 * k2 + 2, :])
                    if k2 == 1:
                        nc.sync.dma_start(x0[:, h:, :], x_d[0][:, h:, :])
                alloc_x(1)
                alloc_x(2)

                # scaled-tril mask blocks (tiny: 2KB/partition, one DMA)
                mk_t = mk_pool.tile([P, NT, P], BF16, tag="mk", name="mk_t")
                nc.sync.dma_start(mk_t[:], mk_d[:])
                mk_ts = [mk_t[:, q, :] for q in range(NT)]

                v_ts = [v_pool.tile([P, C], BF16, tag=f"v{tt}", name=f"vt{tt}")
                        for tt in range(NT)]
                y_ts = [y_pool.tile([P, TH], BF16, tag=f"y{cc}", name=f"yt{cc}")
                        for cc in range(NK)]

                def emit_ph2(tb, cc):
                    # yT[c-tile cc, 128-block q] = v'[q].T @ mk_q — 4 N=128
                    # matmuls whose LDWEIGHTS hide under neighbouring N=512
                    # streams via the PE reorder window
                    psum2 = ps_pool.tile([P, NB], F32, tag="ps2", bufs=ps2_bufs)
                    for i in range(NB // P):
                        q = tb * (NB // P) + i
                        nc.tensor.matmul(
                            psum2[:, i * P:(i + 1) * P],
                            v_ts[q][:, cc * P:(cc + 1) * P],
                            mk_ts[q][:], start=True, stop=True)
                    (nc.scalar.copy if ph2_eng == "scalar"
                     else nc.vector.tensor_copy)(
                        y_ts[cc][:, tb * NB:(tb + 1) * NB], psum2[:])

                # ---- phase 1: v = x' @ Wc.T  (phase-2 groups interleaved
                # into the second half once their v-tiles exist) ----
                for g, (tt, cb) in enumerate(
                        (tt, cb) for tt in range(NT) for cb in range(NTB)):
                    if tt not in x_ts:
                        alloc_x(tt)
                    x_t = x_ts[tt]
                    psum1 = ps_pool.tile([P, NB], F32, tag="ps1", bufs=ps1_bufs)
                    for k in range(NK):
                        nc.tensor.matmul(
                            psum1[:], x_t[:, k, :],
                            wc_ts[k][:, cb * NB:(cb + 1) * NB],
                            start=(k == 0), stop=(k == NK - 1))
                    nc.vector.tensor_copy(v_ts[tt][:, cb * NB:(cb + 1) * NB],
                                          psum1[:])
                    if g >= 8:
                        emit_ph2(0, g - 8)   # needs v[0..3] only

                # wp as one [P, NK, C] tile filled by 2 contiguous DMAs
                wp_t = wp_pool.tile([P, NK, C], BF16, tag="wp", name="wp_t")
                for h in range(2):
                    nc.sync.dma_start(wp_t[:, h * (NK // 2):(h + 1) * (NK // 2), :],
                                      wp_d[:, h * (NK // 2):(h + 1) * (NK // 2), :])
                wp_ts = [wp_t[:, k, :] for k in range(NK)]

                # two ph2(tb=1) groups right away so PE has work while the
                # last ph2(tb=0) ACT copy lands
                emit_ph2(1, 0)
                emit_ph2(1, 1)

                def emit_ph3(tb, dt_, t0, tn):
                    psum3 = ps_pool.tile([P, tn], F32,
                                         tag="ps3" if tn == NB else "ps3s",
                                         bufs=ps3_bufs if tn == NB else 2)
                    for cc in range(NK):
                        nc.tensor.matmul(
                            psum3[:], wp_ts[cc][:, dt_ * P:(dt_ + 1) * P],
                            y_ts[cc][:, tb * NB + t0:tb * NB + t0 + tn],
                            start=(cc == 0), stop=(cc == NK - 1))
                    o_t = o_pool.tile([P, tn], BF16, tag="o")
                    if mix_copies and dt_ % 2:
                        nc.scalar.copy(o_t[:], psum3[:])
                    else:
                        nc.vector.tensor_copy(o_t[:], psum3[:])
                    nc.sync.dma_start(o_d[dt_, :, tb, t0:t0 + tn], o_t[:])

                # ---- phase 3: outT = Wp @ yT, remaining ph2(tb=1) groups
                # interleaved; last group split for a shorter drain tail ----
                for dt_ in range(NK):
                    emit_ph3(0, dt_, 0, NB)
                    if dt_ < 6:
                        emit_ph2(1, dt_ + 2)
                for dt_ in range(NK):
                    if dt_ < NK - 1 or not tail_split:
                        emit_ph3(1, dt_, 0, NB)
                    else:
                        emit_ph3(1, dt_, 0, NB // 2)
                        emit_ph3(1, dt_, NB // 2, NB // 2)

            if bench and repeat > 1:
                UNROLL = next((u for u in (8, 4, 2) if repeat % u == 0), 1)
                with tc.For_i(0, repeat // UNROLL, 1):
                    if wu:
                        warmup()
                    for _u in range(UNROLL):
                        body(with_wu=False)
            else:
                for _rep in range(repeat):
                    body()
            if bench:
                with tc.tile_pool(name="dummy", bufs=1) as d_pool:
                    d_t = d_pool.tile([P, 8], F32)
                    nc.sync.dma_start(d_t[:], din_d[:])
                    nc.sync.dma_start(dout_d[:], d_t[:])

    nc.compile()
    return nc


def _get_program(repeat=1, bench=False, **kw):
    key = ("nc", repeat, bench, tuple(sorted(kw.items())))
    if key not in _CACHE:
        _CACHE[key] = _build(repeat, bench, **kw)
    return _CACHE[key]


def _mask_consts():
    # scaled-tril blocks [p(s), q, t] per sequence-half j:
    # mk_j[s, q, t] = 1/(1024j + 128q + t + 1) if s<=t else 0. Input-independent.
    if "masks" not in _CACHE:
        tri = np.tril(np.ones((P, P), dtype=np.float32)).T  # [s, t], s<=t
        masks = []
        for j in range(2):
            blocks = []
            for q in range(NT):
                t0 = TH * j + P * q
                scale = 1.0 / (np.arange(t0, t0 + P, dtype=np.float32) + 1.0)
                blocks.append(tri * scale[None, :])
            mk = np.stack(blocks, 0)  # [q, s, t]
            masks.append(np.ascontiguousarray(
                mk.transpose(1, 0, 2)).astype(NPBF16))  # [p(s), q, t]
        _CACHE["masks"] = masks
    return _CACHE["masks"]


def _prep_inputs(x, Wc, Wp):
    x = np.ascontiguousarray(np.asarray(x, dtype=np.float32))
    Wc = np.asarray(Wc, dtype=np.float32)
    Wp = np.asarray(Wp, dtype=np.float32)
    B = x.shape[0]

    # Wc.T [k,c] -> [p(k), kt, c];  Wp.T [c,d] -> [p(c), ct, d]
    wc_in = np.ascontiguousarray(
        Wc.T.reshape(NK, P, C).transpose(1, 0, 2)).astype(NPBF16)
    wp_in = np.ascontiguousarray(
        Wp.T.reshape(NK, P, C).transpose(1, 0, 2)).astype(NPBF16)

    masks = _mask_consts()

    in_maps = []
    for core in CORES:
        b, j = divmod(core, 2)
        # prefix-fold: row 0 of each 128-block gets the global cumulative sum
        # of all prior rows of this batch folded in (fp32, before bf16 cast)
        blksum = x[b].reshape(2 * NT, P, C).sum(axis=1)       # [16, C]
        cum = np.cumsum(blksum, axis=0)                        # [16, C]
        xs = x[b, TH * j:TH * (j + 1)].copy()
        for q in range(NT):
            g = NT * j + q
            if g:
                xs[P * q] += cum[g - 1]
        # xs.T [k,t] -> [tt, p(k), kt, t]
        xt = np.ascontiguousarray(
            xs.T.reshape(NK, P, NT, P).transpose(2, 1, 0, 3)).astype(NPBF16)
        in_maps.append({"xt": xt, "wc": wc_in, "wp": wp_in, "mk": masks[j]})
    return in_maps


def _run(x, Wc, Wp, trace=False, repeat=1, **kw):
    nc = _get_program(repeat, **kw)
    in_maps = _prep_inputs(x, Wc, Wp)
    res = run_bass_kernel_spmd(nc, in_maps, CORES, trace=trace)
    B = np.asarray(x).shape[0]
    out = np.empty((B, 2 * TH, C), dtype=np.float32)
    for core in CORES:
        b, j = divmod(core, 2)
        oT = res.results[core]["outT"]            # [dt, p(d), tb, t]
        out[b, TH * j:TH * (j + 1)] = oT.reshape(C, TH).T.astype(np.float32)
    return out, res


def kernel(x, Wc, Wp):
    out, _ = _run(x, Wc, Wp, trace=False)
    return out
